# revision 51
# baseline (speedup 1.0000x reference)
"""Trainium2 Bass kernel for a BasicTransformerBlock (AdaLN + self-attn with
relative position bias + cross-attn + GEGLU FFN), distributed over 8
NeuronCores.

Sharding: core c handles batch b = c//2 and token half h = c%2 (512 of the
1024 tokens of its batch). Token *tiles* (128 tokens each) are permuted
host-side so the core's local tokens are always device tiles 0..3 and the
rel-bias-clipped key tiles are always device tiles 5..7 — one SPMD program is
valid for every core; all per-core variation lives in the input data.

v5 (~511-515us, from 580us):
- fp8e4 DoubleRow matmuls for all weight-stationary GEMMs incl. BOTH GEGLU
  FFN1 halves (weights host-scaled by S=64; residual stream carried at 64x).
- FF1 emits gatedT directly in [di, tok] layout (lhsT = wff1 chunk, rhs =
  n3t) — no PE transposes or PSUM evictions for the 4096-dim inner tensor;
  gatedT carries the S scale and wff2 is host-scaled by 1 to compensate.
- silu(temb) precomputed host-side, shipped fp8 token-major; AdaLN weights
  quarter-major so startup DMAs are contiguous and criticality-ordered.
- eq-LN rsqrt computed as exp(-0.5*ln(var+eps)) and the activation-table
  pass steered so Exp/Ln share one table set: 5 ACT_TABLE_LOADs total
  (was 16-29 at ~2.7us each).
- rel bias applied POST-exp as a bf16 multiply with host-precomputed
  exp(rel) (frees the scores PSUM early; the old PSUM-read add was 2.2us);
  fully-clipped key tiles still use the exp() per-head bias AP.
- q/k stored fp8 (scores matmuls run fp8 non-DR at the same rate as bf16,
  saves 24KB/partition SBUF for the bf16 erel + prefetched stage-D/E
  weights: wada2 scale half + wq2 fetched during self-attention).
- upper x0 tiles (keys-only) shipped bf16; softmax divide via
  reciprocal_approx_fast + ones-matmul partition broadcast; AV HAM-warmer
  matmuls removed (pipeline is tight enough that the PE never downclocks
  there); seam transpose evictions and the LN3 normalize moved to the idle
  Scalar engine (Identity with per-partition scale/bias APs).
- scores PSUM pool deepened to 3 buffers (bg K/V units sequentialized to
  single-tile PSUM usage to free the bank): the PE runs two score tiles
  ahead of the softmax-exp chain instead of one. FFN PSUM pools likewise
  deepened (FF1 ps 4->6, FF2 psw 3->4) so the PE runs further ahead of the
  gelu/mult consumers.

Error budget: rel err 1.714e-02 (gate 2e-2), dominated by fp8 quantization
of the n3 activations feeding FFN1 (both halves); all other fp8 stages
contribute <1e-3 each (measured by per-stage numpy ablation).
"""

import os
import sys

for _p in ("/opt/trn_rl_repo", "/root/.axon_site/_ro/trn_rl_repo"):
    if os.path.isdir(_p) and _p not in sys.path:
        sys.path.insert(0, _p)

import numpy as np
import ml_dtypes

import concourse.bass as bass
import concourse.mybir as mybir
from concourse import bacc
from concourse.tile import TileContext

# Steer the activation-table pass: Exp and Ln alternate in eq-LN (rsqrt is
# computed as exp(-0.5*ln(var))), so the only set allowed to serve them is
# the one containing both. Entries are emptied in place (list order and
# indices must be preserved for walrus's act.json remap).
_orig_get_act_tables = bacc.get_activation_tables


def _steered_act_tables(arch):
    t = dict(_orig_get_act_tables(arch))
    AFT = mybir.ActivationFunctionType
    excl = {AFT.Exp, AFT.Ln}
    if "natural_log_exp_and_others" in t:
        for name in list(t):
            if name != "natural_log_exp_and_others":
                t[name] = t[name] - excl
    return t


bacc.get_activation_tables = _steered_act_tables
from concourse.masks import make_identity

BF = ml_dtypes.bfloat16
F8NP = ml_dtypes.float8_e4m3
F32 = mybir.dt.float32
BF16 = mybir.dt.bfloat16
FP8 = mybir.dt.float8e4
AF = mybir.ActivationFunctionType
OP = mybir.AluOpType
DR = mybir.MatmulPerfMode.DoubleRow

P = 128
D = 1024
T = 1024
NL = 512          # local tokens per core
H = 16
DH = 64
DI = 4096
G = 4
GS = D // G       # 256
MAXREL = 32
EPS = 1e-5
NT = T // P       # 8 token tiles (full batch)
NLT = NL // P     # 4 local token tiles
NC_ = D // P      # 8 dmodel chunks
NSTRAD = 5        # key tiles 0..4 straddle the rel band; 5..7 fully clipped

S = 64.0          # weight / residual scale
SINV = 1.0 / S
EPS_SC = EPS * S * S

# fp8 (DoubleRow) per stage; False = bf16 matmuls for that stage
FP8_ADA = True    # silu(temb) @ w_ada
FP8_QKV = True    # x1t/x2t/enct + q/k/v projections
FP8_AV = True     # es2/v_all + attention AV matmul
FP8_SC = True     # q_all/k_all operands of the scores matmuls kept fp8 (SBUF)
FP8_O = True      # avT + output projection
FP8_FF1 = True    # n3t + first FFN matmul (both halves fp8; err budget checked)
FP8_FF2 = False   # gatedT + second FFN matmul
FP8_FF1A = True   # fp8 only for the linear (a) half of FFN1; gelu half bf16

LEAN = False      # shrink pipeline pools (sim-only validation of bf16 configs)


def _mm_acc(nc, ps, lhs_fn, rhs_fn, fp8, nk=NC_):
    """Accumulating GEMM: fp8 DoubleRow over nk//2 chunk pairs, or bf16 over
    nk chunks. lhs_fn/rhs_fn(c, w) -> AP for chunks [c, c+w)."""
    if fp8:
        for j in range(nk // 2):
            nc.tensor.matmul(ps, lhs_fn(2 * j, 2), rhs_fn(2 * j, 2),
                             start=(j == 0), stop=(j == nk // 2 - 1),
                             perf_mode=DR)
    else:
        for k in range(nk):
            nc.tensor.matmul(ps, lhs_fn(k, 1), rhs_fn(k, 1),
                             start=(k == 0), stop=(k == nk - 1))


# --------------------------------------------------------------------------
# device program
# --------------------------------------------------------------------------

def _ln_normalize(nc, pools, x_ap, out_ap, eps_tile, norm_scalar=False):
    """eq_ln of one [128, 1024] tile: per-group (G=4, 256 wide) mean/var
    normalize. Scale-invariant (input is 64x; eps_tile is 64^2 * eps).
    rsqrt as exp(-0.5*ln(var+eps)) keeps Scalar in the exp table set.
    norm_scalar: run the normalize as Identity(x*rs - mu*rs) on the Scalar
    engine (for stages where Vector is the seam bottleneck)."""
    stats = pools["stats"]
    mvall = stats.tile([P, G, 2], F32, tag="mv")
    for g in range(G):
        st = stats.tile([P, 6], F32, tag="bnst")
        nc.vector.bn_stats(out=st, in_=x_ap[:, g * GS:(g + 1) * GS])
        nc.vector.bn_aggr(out=mvall[:, g, :], in_=st)
    lv = stats.tile([P, G], F32, tag="sd")
    nc.scalar.activation(out=lv, in_=mvall[:, :, 1], func=AF.Ln,
                         bias=eps_tile)
    rs = stats.tile([P, G], F32, tag="rs")
    nc.scalar.activation(out=rs, in_=lv, func=AF.Exp, scale=-0.5)
    if norm_scalar:
        nmr = stats.tile([P, G], F32, tag="nmr")
        nc.vector.tensor_tensor(out=nmr, in0=mvall[:, :, 0], in1=rs,
                                op=OP.mult)
        nc.vector.tensor_scalar_mul(out=nmr, in0=nmr, scalar1=-1.0)
        for g in range(G):
            nc.scalar.activation(
                out=out_ap[:, g * GS:(g + 1) * GS],
                in_=x_ap[:, g * GS:(g + 1) * GS], func=AF.Identity,
                scale=rs[:, g:g + 1], bias=nmr[:, g:g + 1])
        return
    for g in range(G):
        nc.vector.tensor_scalar(
            out=out_ap[:, g * GS:(g + 1) * GS], in0=x_ap[:, g * GS:(g + 1) * GS],
            scalar1=mvall[:, g, 0:1], scalar2=rs[:, g:g + 1],
            op0=OP.subtract, op1=OP.mult)


def _transpose4(nc, pools, src_bf, dst_fn, ident, scale=None, engine="V",
                dst2_fn=None):
    """PE-transpose a [128, W] bf16 tile in 4-chunk batches; one widened
    eviction per batch (a second one on the other engine if dst2_fn).
    dst_fn(c0) -> [128, 4, 128] AP."""
    W = src_bf.shape[-1]
    for c0 in range(0, W // P, 4):
        pt = pools["pst"].tile([P, 4, P], BF16, tag="pst")
        for c in range(4):
            nc.tensor.transpose(pt[:, c, :],
                                src_bf[:, (c0 + c) * P:(c0 + c + 1) * P], ident)
        dst = dst_fn(c0)
        if engine == "V":
            if scale is None:
                nc.vector.tensor_copy(out=dst, in_=pt)
            else:
                nc.vector.tensor_scalar_mul(out=dst, in0=pt, scalar1=scale)
        else:
            nc.scalar.activation(out=dst, in_=pt, func=AF.Copy,
                                 scale=(1.0 if scale is None else scale))
        if dst2_fn is not None:
            if engine == "V":
                nc.scalar.activation(out=dst2_fn(c0), in_=pt, func=AF.Copy)
            else:
                nc.vector.tensor_copy(out=dst2_fn(c0), in_=pt)


def _sl(tile, col_slice):
    def fn(c, w):
        if w == 2:
            return tile[:, c:c + 2, col_slice]
        return tile[:, c, col_slice]
    return fn


def _adaln(nc, pools, n_tiles, x_src, wada_pair, stemb, x1_dst_bf, eps_tile):
    """AdaLN: ss = silu(temb) @ (S*w_ada)^T;
    x1 = eq_ln(x) * (1+ss_lo/S) + ss_hi/S -> bf16 x1_dst_bf[:, t, :]."""
    for t in range(n_tiles):
        ps_sc = pools["psw"].tile([P, D], F32, tag="psw")
        ps_sh = pools["psw"].tile([P, D], F32, tag="psw")

        def st_sl(c, w, t=t):
            if w == 2:
                return stemb[:, t, c:c + 2, :]
            return stemb[:, t, c, :]

        for nb in range(2):
            _mm_acc(nc, ps_sc[:, nb * 512:(nb + 1) * 512], st_sl,
                    _sl(wada_pair[nb], slice(None)), FP8_ADA)
        for nb in range(2):
            _mm_acc(nc, ps_sh[:, nb * 512:(nb + 1) * 512], st_sl,
                    _sl(wada_pair[2 + nb], slice(None)), FP8_ADA)
        scale1p = pools["work"].tile([P, D], BF16, tag="scale1p")
        nc.scalar.activation(out=scale1p, in_=ps_sc, func=AF.Copy,
                             scale=SINV, bias=1.0)
        shift = pools["work"].tile([P, D], BF16, tag="shift")
        nc.scalar.activation(out=shift, in_=ps_sh, func=AF.Copy, scale=SINV)
        n_t = pools["work"].tile([P, D], BF16, tag="n_t")
        _ln_normalize(nc, pools, x_src(t), n_t, eps_tile)
        # x1 elementwise split: mult on DVE, add on the idle gpsimd engine
        nc.vector.tensor_tensor(out=n_t, in0=n_t, in1=scale1p, op=OP.mult)
        nc.gpsimd.tensor_tensor(out=x1_dst_bf[:, t, :], in0=n_t, in1=shift,
                                op=OP.add)


def _q_proj(nc, pools, q_src, wq_sb, q_all):
    for hp in range(8):
        ps_q = pools["ps"].tile([P, 512], F32, tag="ps")
        _mm_acc(nc, ps_q, _sl(wq_sb, slice(hp * P, (hp + 1) * P)),
                _sl(q_src, slice(None)), FP8_QKV)
        nc.scalar.activation(out=q_all[:, hp, :], in_=ps_q, func=AF.Copy,
                             scale=SINV)


def _kv_units(nc, pools, kv_src, wk_sb, wv_sb, k_all, v_all):
    """Closures, each emitting one K- or V-projection chunk (fits the ps
    pool); run inline or interleaved into an attention core."""
    units = []
    for hp in range(8):
        def k_unit(hp=hp):
            wsl = _sl(wk_sb, slice(hp * P, (hp + 1) * P))
            ps_k = pools["ps"].tile([P, 512], F32, tag="ps")
            _mm_acc(nc, ps_k, wsl, _sl(kv_src, slice(0, 512)), FP8_QKV)
            nc.vector.tensor_scalar_mul(out=k_all[:, hp, 0:512], in0=ps_k,
                                        scalar1=SINV)
            ps_k2 = pools["ps"].tile([P, 512], F32, tag="ps")
            _mm_acc(nc, ps_k2, wsl, _sl(kv_src, slice(512, 1024)), FP8_QKV)
            nc.vector.tensor_scalar_mul(out=k_all[:, hp, 512:1024], in0=ps_k2,
                                        scalar1=SINV)
        units.append(k_unit)
    for tt in range(NT):
        def v_unit(tt=tt):
            kv_sl = _sl(kv_src, slice(tt * P, (tt + 1) * P))
            for half in (0, 1):
                psv = pools["ps"].tile([P, 512], F32, tag="ps")
                _mm_acc(nc, psv, kv_sl,
                        _sl(wv_sb, slice(half * 512, (half + 1) * 512)),
                        FP8_QKV)
                src = psv.rearrange("p (q c) -> p q c", c=128)
                dst = v_all[:, tt, half * 640:(half + 1) * 640].rearrange(
                    "p (q c) -> p q c", c=160)
                if half == 0:
                    nc.scalar.activation(out=dst[:, :, 0:64], in_=src[:, :, 0:64],
                                         func=AF.Copy, scale=SINV)
                    nc.scalar.activation(out=dst[:, :, 80:144],
                                         in_=src[:, :, 64:128],
                                         func=AF.Copy, scale=SINV)
                else:
                    nc.vector.tensor_scalar_mul(out=dst[:, :, 0:64],
                                                in0=src[:, :, 0:64], scalar1=SINV)
                    nc.vector.tensor_scalar_mul(out=dst[:, :, 80:144],
                                                in0=src[:, :, 64:128],
                                                scalar1=SINV)
        units.append(v_unit)
    return units


def _qkv(nc, pools, q_src, kv_src, wq_sb, wk_sb, wv_sb, q_all, k_all, v_all):
    """fp8 DoubleRow projections. q_all/k_all bf16 (true scale, 1/S evict);
    v_all [P, NT, 1280] fp8 (true scale): 160-col pair blocks with ones."""
    _q_proj(nc, pools, q_src, wq_sb, q_all)
    # ones columns: one strided memset over the [16, 80]-block view
    ones_view = v_all[:, :, 0:1280].rearrange("p t (k c) -> p t k c", c=80)
    nc.vector.memset(ones_view[:, :, :, 64:65], 1.0)
    for u in _kv_units(nc, pools, kv_src, wk_sb, wv_sb, k_all, v_all):
        u()


def _attention_core(nc, pools, ones128, q_all, k_all, v_all, wo_sb,
                    erel_fn, relc, x_res_src, x_out_dst, bg_units=()):
    """Row-tiled bf16 scores, fp8-DR AV with separate ones-lhsT sums rows,
    softmax denominators inverted with reciprocal_approx_fast after a
    ones-matmul row broadcast, fp8-DR O-projection, residual add into the
    64x fp32 stream."""
    avT = pools["avT"].tile([P, 8, NL], FP8 if FP8_O else BF16, tag="avT")

    erel_tiles = {}

    def fetch_erel(hp):
        if erel_fn is not None and hp < 8:
            erel_tiles[hp] = erel_fn(hp)

    def emit_scores(hp):
        es2 = pools["es"].tile([P, NT, 2, NL], FP8 if FP8_AV else BF16, tag="es")
        erel = erel_tiles.get(hp)
        for tt in range(NT):
            ps_s = pools["psw"].tile([P, 1024], F32, tag="psw")
            nc.tensor.matmul(ps_s[:, 0:512],
                             k_all[0:64, hp, tt * P:(tt + 1) * P],
                             q_all[0:64, hp, :], start=True, stop=True)
            nc.tensor.matmul(ps_s[:, 512:1024],
                             k_all[64:128, hp, tt * P:(tt + 1) * P],
                             q_all[64:128, hp, :], start=True, stop=True)
            if erel_fn is None:
                nc.scalar.activation(out=es2[:, tt, :, :], in_=ps_s,
                                     func=AF.Exp)
            elif tt >= NSTRAD:
                # fully-clipped rel tile: add the clip bias in the exponent
                nc.scalar.activation(out=es2[:, tt, 0, :], in_=ps_s[:, 0:512],
                                     func=AF.Exp, bias=relc[:, 2 * hp:2 * hp + 1])
                nc.scalar.activation(out=es2[:, tt, 1, :], in_=ps_s[:, 512:1024],
                                     func=AF.Exp, bias=relc[:, 2 * hp + 1:2 * hp + 2])
            else:
                # straddling rel tile: exp first (frees the PSUM early), then
                # multiply by the host-precomputed exp(rel) in bf16 on DVE.
                eraw = pools["cwork"].tile([P, 2, NL], BF16, tag="esr")
                nc.scalar.activation(out=eraw, in_=ps_s, func=AF.Exp)
                nc.vector.tensor_tensor(out=es2[:, tt, :, :], in0=eraw,
                                         in1=erel[:, tt, :, :], op=OP.mult)
        return es2

    sums_sbs = {}
    avraws = {}

    def emit_av(hp, es2):
        for hh in range(2):
            ps_av = pools["ps"].tile([P, 512], F32, tag="ps")
            pass  # warmers removed (test)
            c0 = hp * 160 + hh * 80

            def es_sl(c, w, hh=hh):
                if w == 2:
                    return es2[:, c:c + 2, hh, :]
                return es2[:, c, hh, :]

            _mm_acc(nc, ps_av[0:65, :], _sl(v_all, slice(c0, c0 + 65)),
                    es_sl, FP8_AV, nk=NT)
            if hh == 0:
                sums_sb = pools["cwork"].tile([1, 2, NL], BF16, tag="sums")
                sums_sbs[hp] = sums_sb
                avraw = pools["cwork"].tile([P, NL], BF16, tag="avraw")
                avraws[hp] = avraw
                nc.scalar.copy(out=avraw[0:64, :], in_=ps_av[0:64, :])
            else:
                sums_sb = sums_sbs[hp]
                avraw = avraws[hp]
                nc.vector.tensor_copy(out=avraw[64:128, :], in_=ps_av[0:64, :])
            nc.vector.tensor_copy(out=sums_sb[:, hh, :], in_=ps_av[64:65, :])

    def emit_divide(hp):
        sums_sb = sums_sbs.pop(hp)
        avraw = avraws.pop(hp)
        ps_bb = pools["psw"].tile([P, 1024], F32, tag="psw")
        nc.tensor.matmul(ps_bb[:, 0:512], ones128, sums_sb[:, 0, :],
                         start=True, stop=True)
        nc.tensor.matmul(ps_bb[:, 512:1024], ones128, sums_sb[:, 1, :],
                         start=True, stop=True)
        rsb = pools["cw1"].tile([P, 2, NL], F32, tag="rsb")
        nc.vector.reciprocal_approx_fast(out=rsb, in_=ps_bb)
        nc.vector.tensor_tensor(out=avT[0:64, hp, :], in0=avraw[0:64, :],
                                in1=rsb[0:64, 0, :], op=OP.mult)
        nc.vector.tensor_tensor(out=avT[64:128, hp, :],
                                in0=avraw[64:128, :],
                                in1=rsb[64:128, 1, :], op=OP.mult)

    bg = list(bg_units)

    def run_bg(n):
        while n > 0 and bg:
            bg.pop(0)()
            n -= 1

    fetch_erel(0)
    pend_av = None
    pend_div = None
    for hp in range(8):
        fetch_erel(hp + 1)
        es2 = emit_scores(hp)
        if pend_av is not None:
            emit_av(*pend_av)
        elif bg:
            run_bg(2)
        else:
            ps_w = pools["ps"].tile([P, 512], F32, tag="ps")
            for _ in range(6):
                nc.tensor.matmul(ps_w, k_all[:, 0, 0:P], q_all[:, 0, :],
                                 start=True, stop=True)
        run_bg(2)
        if pend_div is not None:
            emit_divide(pend_div)
        pend_div = pend_av[0] if pend_av is not None else None
        pend_av = (hp, es2)
    emit_av(*pend_av)
    if pend_div is not None:
        emit_divide(pend_div)
    emit_divide(7)
    run_bg(len(bg))

    for lt in range(NLT):
        ps_o = pools["psw"].tile([P, 1024], F32, tag="psw")
        for nb in range(2):
            _mm_acc(nc, ps_o[:, nb * 512:(nb + 1) * 512],
                    _sl(avT, slice(lt * P, (lt + 1) * P)),
                    _sl(wo_sb, slice(nb * 512, (nb + 1) * 512)), FP8_O)
        nc.vector.tensor_tensor(out=x_out_dst(lt), in0=ps_o, in1=x_res_src(lt),
                                op=OP.add)


def build_nc(sim_compat=False):
    nc = bacc.Bacc("TRN2", target_bir_lowering=False, debug=False)

    # ---- DRAM parameters (per-core layouts, see prep_core_inputs) ----
    d_x0 = nc.declare_dram_parameter("x0", [P, NLT, D], F32, isOutput=False)
    d_x0hi = nc.declare_dram_parameter("x0hi", [P, NLT, D], BF16, isOutput=False)
    d_stemb = nc.declare_dram_parameter("stemb", [P, NT, NC_, P], FP8 if FP8_ADA else BF16, isOutput=False)
    d_enct = nc.declare_dram_parameter("enct", [P, NC_, T], FP8 if FP8_QKV else BF16, isOutput=False)
    d_erel = nc.declare_dram_parameter("erel", [8, P, NSTRAD, 2, NL], BF16,
                                       isOutput=False)
    d_relc = nc.declare_dram_parameter("relc", [P, H], F32, isOutput=False)
    d_wada1 = nc.declare_dram_parameter("wada1", [P, 4, NC_, 512], FP8 if FP8_ADA else BF16, isOutput=False)
    d_wada2 = nc.declare_dram_parameter("wada2", [P, 4, NC_, 512], FP8 if FP8_ADA else BF16, isOutput=False)
    d_wq1 = nc.declare_dram_parameter("wq1", [P, NC_, D], FP8 if FP8_QKV else BF16, isOutput=False)
    d_wk1 = nc.declare_dram_parameter("wk1", [P, NC_, D], FP8 if FP8_QKV else BF16, isOutput=False)
    d_wv1 = nc.declare_dram_parameter("wv1", [P, NC_, D], FP8 if FP8_QKV else BF16, isOutput=False)
    d_wo1 = nc.declare_dram_parameter("wo1", [P, NC_, D], FP8 if FP8_O else BF16, isOutput=False)
    d_wq2 = nc.declare_dram_parameter("wq2", [P, NC_, D], FP8 if FP8_QKV else BF16, isOutput=False)
    d_wk2 = nc.declare_dram_parameter("wk2", [P, NC_, D], FP8 if FP8_QKV else BF16, isOutput=False)
    d_wv2 = nc.declare_dram_parameter("wv2", [P, NC_, D], FP8 if FP8_QKV else BF16, isOutput=False)
    d_wo2 = nc.declare_dram_parameter("wo2", [P, NC_, D], FP8 if FP8_O else BF16, isOutput=False)
    d_wff1 = nc.declare_dram_parameter(
        "wff1", [P, 8, 2, NC_, 4, P], FP8 if FP8_FF1 else BF16,
        isOutput=False)  # [p, nbh, half, c, jj, dio]
    d_wff2 = nc.declare_dram_parameter("wff2", [P, 32, D], FP8 if FP8_FF2 else BF16, isOutput=False)
    d_out = nc.declare_dram_parameter("out", [P, NLT, D], F32, isOutput=True)

    from contextlib import ExitStack
    with TileContext(nc) as tc, ExitStack() as glob:
        pools = {}
        const = glob.enter_context(tc.tile_pool(name="const", bufs=1))
        pools["stats"] = glob.enter_context(tc.tile_pool(name="stats", bufs=6))

        ident = const.tile([P, P], BF16)
        make_identity(nc, ident)
        eps_tile = const.tile([P, 1], F32)
        nc.vector.memset(eps_tile, EPS_SC)

        ones128 = const.tile([1, P], BF16)
        nc.vector.memset(ones128, 1.0)
        warm = const.tile([P, 1], F32)
        nc.vector.memset(warm, 1.0)
        relc = const.tile([P, H], F32)
        nc.sync.dma_start(out=relc, in_=d_relc[:, :])

        p_xB = glob.enter_context(tc.tile_pool(name="xB_pool", bufs=1))
        xB = p_xB.tile([P, NLT, D], F32)
        p_x2t = glob.enter_context(tc.tile_pool(name="x2t_pool", bufs=1))
        x2t = p_x2t.tile([P, NC_, NL], FP8 if FP8_QKV else BF16)

        xA_stk = ExitStack()         # -> closes after E
        p_xA = xA_stk.enter_context(tc.tile_pool(name="xA_pool", bufs=1))
        xA = p_xA.tile([P, NLT, D], F32)

        # cross-attention K/V tensors: outlive stage C (used in E); filled
        # by background units during self-attention.
        qkv2_stk = ExitStack()       # -> closes after E
        p_qkv2 = qkv2_stk.enter_context(tc.tile_pool(name="qkv2", bufs=1))
        k2_all = p_qkv2.tile([P, 8, T], FP8 if FP8_SC else BF16, tag="k_all")
        v2_all = p_qkv2.tile([P, NT, 1280], FP8 if FP8_AV else BF16, tag="v_all")

        stemb_stk = ExitStack()      # -> closes after D
        p_stemb = stemb_stk.enter_context(tc.tile_pool(name="stemb", bufs=1))
        stemb = p_stemb.tile([P, NT, NC_, P], FP8 if FP8_ADA else BF16)

        wada2_stk = ExitStack()      # -> closes after D
        p_wada2 = wada2_stk.enter_context(tc.tile_pool(name="wada2_pool", bufs=1))
        wada2a = p_wada2.tile([P, 2, NC_, 512], FP8 if FP8_ADA else BF16)

        mid1 = ExitStack()           # x1t, xbase: -> close after C
        p_xbase = mid1.enter_context(tc.tile_pool(name="xbase_pool", bufs=1))
        p_x1t = mid1.enter_context(tc.tile_pool(name="x1t_pool", bufs=1))
        x1t = p_x1t.tile([P, NC_, T], FP8 if FP8_QKV else BF16)
        xbase = p_xbase.tile([P, NLT, D], F32)

        # ---------------- stage A+B: loads, AdaLN1, transpose --------------
        stg = ExitStack()
        pools["psw"] = stg.enter_context(tc.tile_pool(name="bpsw", bufs=3, space="PSUM"))
        pools["pst"] = stg.enter_context(tc.tile_pool(name="bpst", bufs=2, space="PSUM"))
        pools["work"] = stg.enter_context(tc.tile_pool(name="awork", bufs=3))
        p_wada1 = stg.enter_context(tc.tile_pool(name="wada1_pool", bufs=1))
        p_x1s = stg.enter_context(tc.tile_pool(name="x1_stage", bufs=1))
        # startup DMAs: per-chunk/per-tile so the first ss matmul and LN can
        # start as soon as their slices land; spread across engine queues.
        wada1 = p_wada1.tile([P, 4, NC_, 512], FP8 if FP8_ADA else BF16)
        # Criticality-ordered startup: the first ss GEMM needs the wada1
        # scale half + stemb token-slice 0; stream the rest behind it.
        qs = [nc.sync, nc.scalar, nc.gpsimd]
        nc.sync.dma_start(out=wada1[:, 0, 0:2], in_=d_wada1[:, 0, 0:2])
        nc.scalar.dma_start(out=stemb[:, 0], in_=d_stemb[:, 0])
        nc.sync.dma_start(out=wada1[:, 0, 2:], in_=d_wada1[:, 0, 2:])
        nc.scalar.dma_start(out=wada1[:, 1], in_=d_wada1[:, 1])
        nc.gpsimd.dma_start(out=xbase[:, 0, :], in_=d_x0[:, 0, :])
        nc.sync.dma_start(out=wada1[:, 2], in_=d_wada1[:, 2])
        nc.scalar.dma_start(out=wada1[:, 3], in_=d_wada1[:, 3])
        for t in range(1, NT):
            qs[t % 3].dma_start(out=stemb[:, t], in_=d_stemb[:, t])
        for t in range(1, NLT):
            nc.gpsimd.dma_start(out=xbase[:, t, :], in_=d_x0[:, t, :])
        p_xhi = stg.enter_context(tc.tile_pool(name="xhi_pool", bufs=1))
        xhi = p_xhi.tile([P, NLT, D], BF16)
        nc.scalar.dma_start(out=xhi, in_=d_x0hi[:, :, :])
        # warm the exp/ln table set while the DMAs stream (after the trigger
        # instructions so the ~2.7us table load doesn't delay them)
        nc.scalar.activation(out=warm, in_=warm, func=AF.Exp)

        def x0_src(t):
            if t < NLT:
                return xbase[:, t, :]
            return xhi[:, t - NLT, :]

        x1_tiles = p_x1s.tile([P, NT, D], BF16)
        _adaln(nc, pools, NT, x0_src,
               (wada1[:, 0], wada1[:, 1], wada1[:, 2], wada1[:, 3]), stemb,
               x1_tiles, eps_tile)
        for t in range(NT):
            _transpose4(
                nc, pools, x1_tiles[:, t, :],
                lambda c0, t=t: x1t[:, c0:c0 + 4, t * P:(t + 1) * P], ident,
                engine="V" if t % 2 else "A")
        stg.close()

        # ---------------- stage C: self-attention --------------------------
        qkv_stk = ExitStack()
        p_qkv = qkv_stk.enter_context(tc.tile_pool(name="qkv1", bufs=1))
        q_all = p_qkv.tile([P, 8, NL], FP8 if FP8_SC else BF16, tag="q_all")
        k_all = p_qkv.tile([P, 8, T], FP8 if FP8_SC else BF16, tag="k_all")
        v_all = p_qkv.tile([P, NT, 1280], FP8 if FP8_AV else BF16, tag="v_all")
        w2_stk = ExitStack()
        p_enc = w2_stk.enter_context(tc.tile_pool(name="enc_pool", bufs=1))
        p_w2 = w2_stk.enter_context(tc.tile_pool(name="wqkv2", bufs=1))
        enc = p_enc.tile([P, NC_, T], FP8 if FP8_QKV else BF16)
        wk2 = p_w2.tile([P, NC_, D], FP8 if FP8_QKV else BF16, tag="wk")
        wv2 = p_w2.tile([P, NC_, D], FP8 if FP8_QKV else BF16, tag="wv")
        ones_view2 = v2_all[:, :, 0:1280].rearrange("p t (k c) -> p t k c", c=80)
        nc.vector.memset(ones_view2[:, :, :, 64:65], 1.0)

        stg = ExitStack()
        pools["ps"] = stg.enter_context(tc.tile_pool(name="cps", bufs=2, space="PSUM"))
        pools["psw"] = stg.enter_context(tc.tile_pool(name="cpsw", bufs=3, space="PSUM"))
        kv2_units = _kv_units(nc, pools, enc, wk2, wv2, k2_all, v2_all)
        w1_stk = ExitStack()
        p_w1 = w1_stk.enter_context(tc.tile_pool(name="wqkv1", bufs=1))
        wq1 = p_w1.tile([P, NC_, D], FP8 if FP8_QKV else BF16, tag="wq")
        wk1 = p_w1.tile([P, NC_, D], FP8 if FP8_QKV else BF16, tag="wk")
        wv1 = p_w1.tile([P, NC_, D], FP8 if FP8_QKV else BF16, tag="wv")
        # DMA order matters: QKV1 weights are needed first (striped across
        # the three queues), then the cross-attn K/V inputs for the bg units.
        for kc in range(NC_):
            qs[kc % 3].dma_start(out=wq1[:, kc, :], in_=d_wq1[:, kc, :])
            qs[(kc + 1) % 3].dma_start(out=wk1[:, kc, :], in_=d_wk1[:, kc, :])
            qs[(kc + 2) % 3].dma_start(out=wv1[:, kc, :], in_=d_wv1[:, kc, :])
        nc.sync.dma_start(out=enc, in_=d_enct[:, :, :])
        nc.scalar.dma_start(out=wk2, in_=d_wk2[:, :, :])
        nc.gpsimd.dma_start(out=wv2, in_=d_wv2[:, :, :])
        # AdaLN2 scale-half prefetch (used at the C->D seam)
        nc.gpsimd.dma_start(out=wada2a, in_=d_wada2[:, 0:2])
        _qkv(nc, pools, x1t[:, :, 0:NL], x1t, wq1, wk1, wv1,
             q_all, k_all, v_all)
        w1_stk.close()

        p_wo1 = stg.enter_context(tc.tile_pool(name="wo1_pool", bufs=1))
        p_rel = stg.enter_context(tc.tile_pool(name="rel_pool", bufs=1 if LEAN else 2))
        pools["es"] = stg.enter_context(tc.tile_pool(name="es_pool", bufs=1 if LEAN else 2))
        pools["cwork"] = stg.enter_context(tc.tile_pool(name="cwork", bufs=1 if LEAN else 2))
        pools["cw1"] = stg.enter_context(tc.tile_pool(name="cw1", bufs=1))
        pools["avT"] = stg.enter_context(tc.tile_pool(name="avT_pool", bufs=1))
        wo1 = p_wo1.tile([P, NC_, D], FP8 if FP8_O else BF16)
        nc.sync.dma_start(out=wo1, in_=d_wo1[:, :, :])

        def erel_fn(hp):
            er = p_rel.tile([P, NSTRAD, 2, NL], BF16, tag="erel")
            nc.sync.dma_start(out=er, in_=d_erel[hp])
            return er

        _attention_core(nc, pools, ones128, q_all, k_all, v_all, wo1,
                        erel_fn, relc,
                        x_res_src=lambda lt: xbase[:, lt, :],
                        x_out_dst=lambda lt: xA[:, lt, :],
                        bg_units=kv2_units)
        stg.close()
        w2_stk.close()
        qkv_stk.close()
        mid1.close()

        # ---------------- stage D: AdaLN2 + transpose ----------------------
        wq2_stk = ExitStack()        # -> closes after E
        p_w2q = wq2_stk.enter_context(tc.tile_pool(name="wq2_pool", bufs=1))
        wq2 = p_w2q.tile([P, NC_, D], FP8 if FP8_QKV else BF16, tag="wq")
        nc.scalar.dma_start(out=wq2, in_=d_wq2[:, :, :])
        stg = ExitStack()
        pools["psw"] = stg.enter_context(tc.tile_pool(name="dpsw", bufs=3, space="PSUM"))
        pools["pst"] = stg.enter_context(tc.tile_pool(name="dpst", bufs=2, space="PSUM"))
        pools["work"] = stg.enter_context(tc.tile_pool(name="dwork", bufs=2))
        p_wada2b = stg.enter_context(tc.tile_pool(name="wada2b_pool", bufs=1))
        wada2b = p_wada2b.tile([P, 2, NC_, 512], FP8 if FP8_ADA else BF16)
        nc.gpsimd.dma_start(out=wada2b, in_=d_wada2[:, 2:4])
        p_x2s = stg.enter_context(tc.tile_pool(name="x2_stage", bufs=1))
        x2_tiles = p_x2s.tile([P, NLT, D], BF16)
        _adaln(nc, pools, NLT, lambda t: xA[:, t, :],
               (wada2a[:, 0], wada2a[:, 1], wada2b[:, 0], wada2b[:, 1]),
               stemb, x2_tiles, eps_tile)
        for t in range(NLT):
            _transpose4(
                nc, pools, x2_tiles[:, t, :],
                lambda c0, t=t: x2t[:, c0:c0 + 4, t * P:(t + 1) * P], ident,
                engine="A")
        stg.close()

        # ---------------- stage E: cross-attention (Q + core) --------------
        stg = ExitStack()
        pools["ps"] = stg.enter_context(tc.tile_pool(name="eps", bufs=2, space="PSUM"))
        pools["psw"] = stg.enter_context(tc.tile_pool(name="epsw", bufs=3, space="PSUM"))
        p_q2 = stg.enter_context(tc.tile_pool(name="q2_pool", bufs=1))
        q2_all = p_q2.tile([P, 8, NL], FP8 if FP8_SC else BF16, tag="q_all")
        ps_w = pools["ps"].tile([P, 512], F32, tag="ps")
        for _ in range(4):
            nc.tensor.matmul(ps_w, k2_all[:, 0, 0:P], k2_all[:, 0, 0:512],
                             start=True, stop=True)
        _q_proj(nc, pools, x2t, wq2, q2_all)

        p_wo2 = stg.enter_context(tc.tile_pool(name="wo2_pool", bufs=1))
        pools["es"] = stg.enter_context(tc.tile_pool(name="es2_pool", bufs=1 if LEAN else 2))
        pools["cwork"] = stg.enter_context(tc.tile_pool(name="cwork2", bufs=1 if LEAN else 3))
        pools["cw1"] = stg.enter_context(tc.tile_pool(name="cw12", bufs=1))
        pools["avT"] = stg.enter_context(tc.tile_pool(name="avT2_pool", bufs=1))
        wo2 = p_wo2.tile([P, NC_, D], FP8 if FP8_O else BF16)
        nc.sync.dma_start(out=wo2, in_=d_wo2[:, :, :])

        _attention_core(nc, pools, ones128, q2_all, k2_all, v2_all, wo2,
                        None, relc,
                        x_res_src=lambda lt: xA[:, lt, :],
                        x_out_dst=lambda lt: xB[:, lt, :])
        stg.close()
        wq2_stk.close()
        wada2_stk.close()
        stemb_stk.close()
        qkv2_stk.close()
        xA_stk.close()

        # ---------------- stage F: eq-LN + GEGLU FFN -----------------------
        stg = ExitStack()
        f1_psum = ExitStack()
        pools["ps"] = f1_psum.enter_context(tc.tile_pool(name="fps", bufs=6, space="PSUM"))
        pools["pst"] = f1_psum.enter_context(tc.tile_pool(name="fpst", bufs=2, space="PSUM"))
        p_n3t = stg.enter_context(tc.tile_pool(name="n3t_pool", bufs=1))
        p_gT = stg.enter_context(tc.tile_pool(name="gatedT_pool", bufs=1))
        p_wff1 = stg.enter_context(tc.tile_pool(name="wff1_pool", bufs=2))
        p_wff2 = stg.enter_context(tc.tile_pool(name="wff2_pool", bufs=1))
        p_fw = stg.enter_context(tc.tile_pool(name="fwork", bufs=3))
        wff1_tiles = {}

        def fetch_wff1(nbh):
            if nbh < 8:
                wa = p_wff1.tile([P, NC_, 4, P], FP8 if FP8_FF1 else BF16,
                                 tag="wff1")
                wg = p_wff1.tile([P, NC_, 4, P], FP8 if FP8_FF1 else BF16,
                                 tag="wff1g")
                nc.sync.dma_start(out=wa, in_=d_wff1[:, nbh, 0])
                nc.scalar.dma_start(out=wg, in_=d_wff1[:, nbh, 1])
                wff1_tiles[nbh] = (wa, wg)

        fetch_wff1(0)
        fetch_wff1(1)
        n3t = p_n3t.tile([P, NC_, NL], FP8 if FP8_FF1 else BF16)
        n3t8 = None
        if FP8_FF1A and not FP8_FF1:
            n3t8 = p_n3t.tile([P, NC_, NL], FP8, name="n3t8")
        for t in range(NLT):
            n3 = p_fw.tile([P, D], BF16, tag="n3")
            _ln_normalize(nc, pools, xB[:, t, :], n3, eps_tile,
                          norm_scalar=True)
            _transpose4(
                nc, pools, n3,
                lambda c0, t=t: n3t[:, c0:c0 + 4, t * P:(t + 1) * P],
                ident, engine="A",
                dst2_fn=(None if n3t8 is None else
                         (lambda c0, t=t: n3t8[:, c0:c0 + 4, t * P:(t + 1) * P])))
        # gatedT produced directly in [di, tok] layout (lhsT = wff1 chunk,
        # rhs = n3t): no transposes. Carries the S scale; wff2 is host-scaled
        # by 1 so FF2 output stays at S like the residual stream.
        gatedT = p_gT.tile([P, 32, NL], FP8 if FP8_FF2 else BF16)
        wff2 = p_wff2.tile([P, 32, D], FP8 if FP8_FF2 else BF16)
        nc.sync.dma_start(out=wff2, in_=d_wff2[:, :, :])
        for nbh in range(8):
            fetch_wff1(nbh + 2)
            wa, wg = wff1_tiles.pop(nbh)
            for jj in range(4):
                j = nbh * 4 + jj
                ps_a = pools["ps"].tile([P, 512], F32, tag="ps")
                ps_g = pools["ps"].tile([P, 512], F32, tag="ps")

                def wa_sl(c, w, jj=jj):
                    if w == 2:
                        return wa[:, c:c + 2, jj, :]
                    return wa[:, c, jj, :]

                def wg_sl(c, w, jj=jj):
                    if w == 2:
                        return wg[:, c:c + 2, jj, :]
                    return wg[:, c, jj, :]

                _mm_acc(nc, ps_a, wa_sl, _sl(n3t8 if n3t8 is not None else n3t,
                                             slice(None)),
                        FP8_FF1 or FP8_FF1A)
                _mm_acc(nc, ps_g, wg_sl, _sl(n3t, slice(None)), FP8_FF1)
                g_bf = p_fw.tile([P, 512], BF16, tag="g_bf")
                if sim_compat:
                    # CoreSim has no Gelu table: use x*sigmoid(1.702x) and
                    # compare against the same formula host-side.
                    graw = p_fw.tile([P, 512], F32, tag="graw")
                    nc.scalar.activation(out=graw, in_=ps_g, func=AF.Copy,
                                         scale=SINV)
                    nc.scalar.activation(out=g_bf, in_=ps_g, func=AF.Sigmoid,
                                         scale=1.702 * SINV)
                    nc.vector.tensor_tensor(out=g_bf, in0=g_bf, in1=graw,
                                            op=OP.mult)
                else:
                    nc.scalar.activation(out=g_bf, in_=ps_g, func=AF.Gelu,
                                         scale=SINV)
                nc.vector.tensor_tensor(out=gatedT[:, j, :], in0=ps_a,
                                        in1=g_bf, op=OP.mult)
        f1_psum.close()
        f2_psum = ExitStack()
        pools["psw"] = f2_psum.enter_context(tc.tile_pool(name="fpsw", bufs=4, space="PSUM"))
        for lt in range(NLT):
            ps2 = pools["psw"].tile([P, 1024], F32, tag="psw")
            for nb in range(2):
                _mm_acc(nc, ps2[:, nb * 512:(nb + 1) * 512],
                        _sl(gatedT, slice(lt * P, (lt + 1) * P)),
                        _sl(wff2, slice(nb * 512, (nb + 1) * 512)),
                        FP8_FF2, nk=32)
            o64 = p_fw.tile([P, D], F32, tag="o64")
            for nb in range(2):
                cs = slice(nb * 512, (nb + 1) * 512)
                nc.vector.tensor_tensor(out=o64[:, cs], in0=ps2[:, cs],
                                        in1=xB[:, lt, cs], op=OP.add)
                nc.sync.dma_start(out=d_out[:, lt, cs], in_=o64[:, cs])
        f2_psum.close()
        stg.close()
    return nc


# --------------------------------------------------------------------------
# host-side input preparation
# --------------------------------------------------------------------------

def _chunk_w(w_t, n_chunks, fp8=True, scale=None):
    """(D_in, N) -> [128, n_chunks, N] with [p, c, n] = scale*w_t[c*128+p, n]."""
    D_in, N = w_t.shape
    sc = S if scale is None else scale
    return np.ascontiguousarray(
        (w_t * sc).reshape(n_chunks, P, N).transpose(1, 0, 2)).astype(
            F8NP if fp8 else BF)


def prep_core_inputs(core, inputs):
    b, h = core // 2, core % 2
    hs = inputs["hidden_states"][b]          # (1024, 1024) f32
    enc = inputs["encoder_hidden_states"][b]
    temb = inputs["temb"][b * T:(b + 1) * T]

    # device tile tt -> global tile perm[tt]; local query tiles first, then
    # the straddling key tile, then the fully-clipped key tiles.
    perm = [0, 1, 2, 3, 4, 5, 6, 7] if h == 0 else [4, 5, 6, 7, 3, 0, 1, 2]

    x0a = hs.reshape(NT, P, D)[perm].transpose(1, 0, 2)  # [p, tt, d]
    x0a = x0a * S
    x0 = np.ascontiguousarray(x0a[:, 0:NLT]).astype(np.float32)
    x0hi = np.ascontiguousarray(x0a[:, NLT:]).astype(BF)

    temb_perm = temb.reshape(NT, P, D)[perm].reshape(T, D)  # permuted tokens
    st64 = temb_perm.astype(np.float64)
    st64 = st64 / (1.0 + np.exp(-st64))                     # silu, host-side
    # [p, t, c, tp] = silu(temb)[t*128+tp, c*128+p]
    stemb = np.ascontiguousarray(
        st64.reshape(NT, P, NC_, P).transpose(3, 0, 2, 1)).astype(
            F8NP if FP8_ADA else BF)

    enct = np.ascontiguousarray(
        enc.T.reshape(NC_, P, T).transpose(1, 0, 2)).astype(
            F8NP if FP8_QKV else BF)

    # exp of the relative bias for the straddling key tiles (device 0..4),
    # interleaved per head pair: erel[hp, p, tt, hh, u]
    rel = np.asarray(inputs["rel_bias"], np.float64)      # (16, 65)
    qg = h * NL
    uu = np.arange(NL)[None, None, :]
    pp2 = np.arange(P)[:, None, None]
    k0g = (np.array(perm[:NSTRAD]) * P)[None, :, None]
    delta = np.clip((qg + uu) - (k0g + pp2), -MAXREL, MAXREL) + MAXREL
    er = np.exp(rel[:, delta])                            # (16, P, 5, NL)
    erel = er.reshape(8, 2, P, NSTRAD, NL).transpose(0, 2, 3, 1, 4)
    # clip-side bias for the fully-clipped tiles (5..7): h=0 cores clip low,
    # h=1 cores clip high.
    relc = np.broadcast_to(rel[:, 0 if h == 0 else 2 * MAXREL].astype(np.float32),
                           (P, H))

    out = {
        "x0": x0, "x0hi": x0hi, "stemb": stemb, "enct": enct,
        "erel": np.ascontiguousarray(erel).astype(BF),
        "relc": np.ascontiguousarray(relc).astype(np.float32),
        # [p, q, c, n] = S*w_ada[q*512+n, c*128+p]
        "wada1": np.ascontiguousarray(
            (np.asarray(inputs["w_ada1"]) * S)
            .reshape(4, 512, NC_, P).transpose(3, 0, 2, 1)).astype(
                F8NP if FP8_ADA else BF),
        "wada2": np.ascontiguousarray(
            (np.asarray(inputs["w_ada2"]) * S)
            .reshape(4, 512, NC_, P).transpose(3, 0, 2, 1)).astype(
                F8NP if FP8_ADA else BF),
        "wq1": _chunk_w(inputs["wq1"].T / (DH ** 0.5), NC_, FP8_QKV),
        "wk1": _chunk_w(inputs["wk1"].T, NC_, FP8_QKV),
        "wv1": _chunk_w(inputs["wv1"].T, NC_, FP8_QKV),
        "wo1": _chunk_w(inputs["wo1"].T, NC_, FP8_O),
        "wq2": _chunk_w(inputs["wq2"].T / (DH ** 0.5), NC_, FP8_QKV),
        "wk2": _chunk_w(inputs["wk2"].T, NC_, FP8_QKV),
        "wv2": _chunk_w(inputs["wv2"].T, NC_, FP8_QKV),
        "wo2": _chunk_w(inputs["wo2"].T, NC_, FP8_O),
        # wff1 (lhsT): [p, nbh(8), half, c(8), jj(4), dio(128)]
        #            = S*w_ff1[half*DI + (nbh*4+jj)*128 + dio, c*128+p]
        "wff1": np.ascontiguousarray(
            (np.asarray(inputs["w_ff1"]) * S)
            .reshape(2, 8, 4, P, NC_, P).transpose(5, 1, 0, 4, 2, 3)).astype(
                F8NP if FP8_FF1 else BF),
        "wff2": _chunk_w(inputs["w_ff2"].T, 32, FP8_FF2, scale=1.0),
    }
    return out


def check_zero_biases(inputs):
    for k in ("b_ada1", "b_ada2", "bo1", "bo2", "b_ff1", "b_ff2"):
        if np.any(np.asarray(inputs[k])):
            raise NotImplementedError(
                f"bias {k} is nonzero; this kernel build assumes zero biases")


_NC_CACHE = []


def kernel(**inputs):
    inputs = {k: np.asarray(v) for k, v in inputs.items()}
    check_zero_biases(inputs)
    from concourse.bass_utils import run_bass_kernel_spmd
    if not _NC_CACHE:
        nc = build_nc()
        nc.compile()
        _NC_CACHE.append(nc)
    nc = _NC_CACHE[0]
    in_maps = [prep_core_inputs(c, inputs) for c in range(8)]
    res = run_bass_kernel_spmd(nc, in_maps, list(range(8)))
    B = inputs["hidden_states"].shape[0]
    out = np.empty((B, T, D), np.float32)
    for c in range(8):
        b, h = c // 2, c % 2
        o = res.results[c]["out"]            # [128, 4, 1024], 64x scale
        out[b, h * NL:(h + 1) * NL] = \
            o.transpose(1, 0, 2).reshape(NL, D) * np.float32(SINV)
    return out

